# revision 30
# baseline (speedup 1.0000x reference)
"""MiniSTU Trainium2 kernel (8 NeuronCores, Bass/Tile) — v2.

Math: y[b,l,o] = sum_g sum_{t<=l} phi_eff_g[l-t] * (x[b,t] @ M_g)[o]
over g in 48 (filter k, sign) pairs; filter-dim sharded 6 pairs/core,
per-core partials summed on host.

v2 structure (per 128-seq tile c, all DoubleRow fp8 where marked):
  s1     Z[c+1] = x @ (64 M) in fp16, PSUM f32 [two 3-bank triples]
  evac   Z8h = e4m3(psum/4) (~16 Z);  Z8l = psum/4 - Z8h  (Z ~ (Z8h+Z8l)/16
         to ~2^-8 relative: fp16-grade from two fp8 tiles)
  proj   W[c] = P8^T Z8h  — DR-fp8, pair-packed strips (ranks 16,16,16,16,
         32,32 by ascending filter sigma; strips at psum rows 0/32/64)
  far    d>=2 via W-pair tiles [128,2,512] ({2j,2j+1}): one DR-fp8 mm per
         pair-source with G-pair lhsT [G_D; G_{D-1}]; even-c remainder
         single via [G_2; 0].  G fit by lsq against the quantized P8.
  d1,d0  dense, 3-term split-fp8 DR pair-packed: T8h Z8h + T8h Z8l +
         T8l Z8h  (T8l = sT*T - T8h; dropped T8l Z8l ~ 2^-8)
  Far field carries ~13% of output amplitude -> pure-fp8 there is ~2.6e-3
  end-to-end; dense paths are ~2^-8-accurate.  No W2/W3 hierarchy.
"""

import numpy as np
import ml_dtypes

import concourse.bass as bass
import concourse.tile as tile
from concourse import mybir
from concourse.bass_utils import run_bass_kernel_spmd

L = 2048
K = 24
I = 256
O = 256
B = 2
TS = 128
CT = L // TS      # 16 sequence tiles
NP = 6            # (k, sign) pairs per core
NPP = NP // 2
N_CORES = 8
RANKS = (16, 16, 16, 16, 32, 32)   # per in-core pair (sigma ascending)
SOFF = (0, 32, 64)                 # proj strip offsets per pp
SW = (32, 32, 64)                  # strip widths per pp
F32 = mybir.dt.float32
F16 = mybir.dt.float16
FP8 = mybir.dt.float8e4
E4 = ml_dtypes.float8_e4m3
DR = mybir.MatmulPerfMode.DoubleRow

S_T = 16.0        # T-block scale
S_P = 4.0         # P8 = q8(4 P); proj psum = 64 * (P^T Z); W8 = cast(psum)
S_GF = 16.0       # G8 = q8(16 * lsq(P8, fam)); G8^T W8 = 256 * y_far
Y_SCALE = 1.0 / 256.0   # y-psum = 256 * y (applied in _gather)


# ---------------------------------------------------------------------------
# Workarounds for this container's walrus: it rejects any instruction that
# carries more than one sync-wait command.
# ---------------------------------------------------------------------------

def _prune_init_barrier(nc):
    for f in nc.m.functions:
        for blk in f.blocks:
            if blk.name != "main":
                continue
            keep = []
            for inst in blk.instructions:
                nm = type(inst).__name__
                if nm in ("InstMemset", "InstDrain", "InstEventSemaphore"):
                    continue
                keep.append(inst)
            blk.instructions = keep


def _split_sync_waits(nc, max_waits=1):
    for f in nc.m.functions:
        for blk in f.blocks:
            insts = list(blk.instructions)
            out = []
            changed = False
            for inst in insts:
                si = getattr(inst, "sync_info", None)
                waits = list(si.on_wait) if si is not None else []
                if len(waits) > max_waits:
                    changed = True
                    extra, keep = waits[:-max_waits], waits[-max_waits:]
                    for j in range(0, len(extra), max_waits):
                        nop = mybir.InstNoOp(
                            name=nc.get_next_instruction_name(), ins=[], outs=[]
                        )
                        nop.engine = inst.engine
                        nop.sync_info = mybir.SyncInfo(
                            on_wait=extra[j : j + max_waits], on_update=[]
                        )
                        out.append(nop)
                    inst.sync_info = mybir.SyncInfo(
                        on_wait=keep, on_update=list(si.on_update)
                    )
                out.append(inst)
            if changed:
                blk.instructions = out


class _TC(tile.TileContext):
    """TileContext whose tail drain skips the global barrier."""

    def _drain_and_barrier(self, tick_clock, wait_clock):
        nc = self.nc
        nc.sync.drain()
        assert self.sems is not None
        popped = nc._tile_sem_poison_stack.pop()
        assert popped is self._sem_poison


# ---------------------------------------------------------------------------
# Device program
# ---------------------------------------------------------------------------

def _zp(zhtiles, cp, pp, b):
    """zh8(cp, pp, b) as [t, 2(p), 256] — proj rhs."""
    return zhtiles[cp][pp][:, b]


def _zd(ztiles, cp, p):
    """Z16(cp, pair p) as [t, 2(b), 256] — dense b-fused rhs."""
    return ztiles[cp][p // 2][:, :, p % 2]


def _build_nc():
    nc = bass.Bass("TRN2", target_bir_lowering=False, debug=False,
                   num_devices=N_CORES)
    # x^T per seq tile: [cp, i', (b, ic, t)] fp16
    x16_d = nc.dram_tensor("x16", [CT, TS, B * 2 * TS], F16,
                           kind="ExternalInput")
    # s1 rhs, 64*M: [pp, ic, i', (p0 o | p1 o)] fp16
    m16_d = nc.dram_tensor("m16", [NPP, 2, TS, 2 * O], F16,
                           kind="ExternalInput")
    # dense T lhsT: [d(0,1), t, (p, l)] fp16, values 4*T
    t16_d = nc.dram_tensor("t16", [2, TS, NP * TS], F16,
                           kind="ExternalInput")
    # proj basis pack: [t, (sub, strip-cols)] fp8
    p8_d = nc.dram_tensor("p8", [TS, 2 * TS], FP8, kind="ExternalInput")
    # far G pair-lhsT: [14, wrow, (sub, l)]: idx 0 = [G2; 0], idx D-2 = [G_D; G_{D-1}]
    g8_d = nc.dram_tensor("g8", [14, TS, 2 * TS], FP8, kind="ExternalInput")
    # per-core partial output: [c, t, (b, o)] f32
    yp_d = nc.dram_tensor("yp", [CT, TS, B * O], F32, kind="ExternalOutput")

    with _TC(nc) as tc:
        with (
            tc.tile_pool(name="const", bufs=1) as cpool,
            tc.tile_pool(name="z16", bufs=8) as z16p,
            tc.tile_pool(name="z8h", bufs=4) as z8hp,
            tc.tile_pool(name="wstg", bufs=2) as wsp,
            tc.tile_pool(name="ystage", bufs=4) as ypool,
            tc.tile_pool(name="psz", bufs=1, space="PSUM") as psz,
            tc.tile_pool(name="psp", bufs=1, space="PSUM") as psp,
            tc.tile_pool(name="psy", bufs=1, space="PSUM") as psy,
        ):
            xs = [cpool.tile([TS, B * 2 * TS], F16, tag=f"x{cp}",
                             name=f"x{cp}") for cp in range(CT)]
            ms = [[cpool.tile([TS, 2 * O], F16, tag=f"m{pp}{ic}",
                              name=f"m{pp}{ic}") for ic in range(2)]
                  for pp in range(NPP)]
            t16 = [cpool.tile([TS, NP * TS], F16, tag=f"t{d}", name=f"t{d}")
                   for d in range(2)]
            p8t = cpool.tile([TS, 2, TS], FP8, tag="p8", name="p8t")
            g8t = [cpool.tile([TS, 2, TS], FP8, tag=f"g{i}", name=f"g{i}")
                   for i in range(14)]
            wt = [cpool.tile([TS, 2, B * O], FP8, tag=f"w{j}", name=f"w{j}")
                  for j in range(CT // 2 - 1)]   # {14,15} never read

            # ---- head DMAs: first-needed tiles, one per queue engine
            nc.sync.dma_start(ms[0][0][:], m16_d[0, 0])
            nc.gpsimd.dma_start(xs[0][:], x16_d[0])
            nc.sync.dma_start(ms[0][1][:], m16_d[0, 1])
            nc.gpsimd.dma_start(ms[1][0][:], m16_d[1, 0])
            nc.sync.dma_start(ms[1][1][:], m16_d[1, 1])
            nc.gpsimd.dma_start(ms[2][0][:], m16_d[2, 0])
            nc.sync.dma_start(ms[2][1][:], m16_d[2, 1])
            nc.gpsimd.dma_start(xs[1][:], x16_d[1])
            nc.sync.dma_start(t16[0][:], t16_d[0])
            nc.gpsimd.dma_start(t16[1][:], t16_d[1])
            nc.sync.dma_start(p8t[:], p8_d[:])
            nc.gpsimd.dma_start(xs[2][:], x16_d[2])

            ztiles = {}    # cp -> (Z16_A, Z16_B) [TS, 3, 2, 256] fp16
            zhtiles = {}   # cp -> (zh8_A, zh8_B) fp8

            def stage1_pp(sp, pp):
                """Z[sp] pair pp: 4 mms into one [t, 2(b), 2(p), 256] psum
                pair, then immediate Z16 + zh8 evacuation."""
                pz = psz.tile([TS, 2, 2, 256], F32, tag=f"zP{pp}",
                              name=f"zP{pp}")
                for b in range(B):
                    for ic in range(2):
                        lhs = xs[sp][:, (b * 2 + ic) * TS:(b * 2 + ic + 1) * TS]
                        nc.tensor.matmul(
                            pz[:, b], lhs, ms[pp][ic][:],
                            start=(ic == 0), stop=(ic == 1),
                        )
                z16 = z16p.tile([TS, 2, 2, 256], F16, tag="z16",
                                name=f"z16_{pp}")
                zh8 = z8hp.tile([TS, 2, 2, 256], FP8, tag="zh",
                                name=f"zh_{pp}")
                if pp % 2 == 0:
                    nc.scalar.copy(z16[:], pz[:])
                    nc.vector.tensor_scalar_mul(zh8[:], z16[:], 0.25)
                else:
                    nc.vector.tensor_copy(z16[:], pz[:])
                    nc.scalar.activation(zh8[:], z16[:],
                                         mybir.ActivationFunctionType.Identity,
                                         scale=0.25)
                ztiles.setdefault(sp, [None] * NPP)[pp] = z16
                zhtiles.setdefault(sp, [None] * NPP)[pp] = zh8

            def proj_strip(s, pp):
                """One pp's W-strip: 1 b-fused DR mm -> 1-bank psum -> fp8
                cast -> partition-shifting DMA into the wt pair tile."""
                pw = psp.tile([64, B * O], F32, tag="pw", name="pw")
                rhs = zhtiles[s][pp][:].rearrange("t b p o -> t p b o")
                nc.tensor.matmul(
                    pw[0:SW[pp], :],
                    p8t[:, :, SOFF[pp]:SOFF[pp] + SW[pp]],
                    rhs, start=True, stop=True, perf_mode=DR,
                )
                stg = wsp.tile([64, B * O], FP8, tag="stg", name="stg")
                (nc.scalar.copy if pp % 2 == 0 else nc.vector.tensor_copy)(
                    stg[0:SW[pp], :], pw[0:SW[pp], :])
                (nc.sync if pp % 2 else nc.gpsimd).dma_start(
                    wt[s // 2][SOFF[pp]:SOFF[pp] + SW[pp], s % 2],
                    stg[0:SW[pp], :])

            for pp in range(NPP):
                stage1_pp(0, pp)
            for s in range(CT):
                c = s
                # prefetch
                if s + 3 < CT:
                    nc.gpsimd.dma_start(xs[s + 3][:], x16_d[s + 3])
                if s < 7:
                    nc.sync.dma_start(g8t[2 * s][:], g8_d[2 * s])
                    nc.gpsimd.dma_start(g8t[2 * s + 1][:], g8_d[2 * s + 1])

                # proj strips interleave with per-pp s1 chunks; each strip's
                # cast hides behind the next s1 chunk (pw ring is 1 bank).
                # At s=0 zh8[0] lands late (head): s1 chunks go first.
                do_proj = s <= 2 * (len(wt) - 1) + 1
                for pp in range(NPP):
                    if do_proj and s > 0:
                        proj_strip(s, pp)
                    if s + 1 < CT:
                        stage1_pp(s + 1, pp)
                if s == 0:
                    for pp in range(NPP):
                        proj_strip(0, pp)

                # ---- stage 2: output tile c, one PSUM group.  Dense d1/d0
                # fp16 b-fused mms (N=512) interleaved with far fp8-DR pair
                # mms so the G weight loads hide behind the dense runs. ----
                yt = psy.tile([TS, B * O], F32, tag="yt", name="yt")
                jp = (c - 3) // 2 + 1 if c >= 3 else 0   # full pairs
                single = (c >= 2 and c % 2 == 0)

                far = [(g8t[c - 2 * j - 2], wt[j]) for j in range(jp)]
                if single:                   # leftover tile c-2 at D=2
                    far.append((g8t[0], wt[(c - 2) // 2]))
                dense = []
                for dd in ((1, 0) if c >= 1 else (0,)):
                    for p in range(NP):
                        dense.append((
                            t16[dd][:, p * TS:(p + 1) * TS],
                            _zd(ztiles, c - dd, p),
                        ))
                n_mm = len(far) + len(dense)
                i_mm = 0
                while far or dense:
                    if far:
                        g, w = far.pop(0)
                        nc.tensor.matmul(yt[:], g[:], w[:],
                                         start=(i_mm == 0),
                                         stop=(i_mm == n_mm - 1),
                                         perf_mode=DR)
                        i_mm += 1
                    for _ in range(3):
                        if not dense:
                            break
                        th, zr = dense.pop(0)
                        nc.tensor.matmul(yt[:], th, zr,
                                         start=(i_mm == 0),
                                         stop=(i_mm == n_mm - 1))
                        i_mm += 1
                assert i_mm == n_mm
                yst = ypool.tile([TS, B * O], F32, tag="yst", name=f"yst{c}")
                nc.vector.tensor_scalar_mul(yst[:, :O], yt[:, :O], Y_SCALE)
                nc.scalar.activation(yst[:, O:], yt[:, O:],
                                     mybir.ActivationFunctionType.Identity,
                                     scale=Y_SCALE)
                (nc.sync if c % 2 else nc.gpsimd).dma_start(yp_d[c], yst[:])

    _prune_init_barrier(nc)
    _split_sync_waits(nc)
    return nc


# ---------------------------------------------------------------------------
# Host side: prep, sharding, gather
# ---------------------------------------------------------------------------

def _build_toeplitz(phi_eff):
    """tb[d, t, l] = phi_eff[d*TS + l - t] (0 where index < 0)."""
    pad = np.zeros(L + TS - 1, np.float64)
    pad[TS - 1:] = phi_eff
    d = np.arange(CT)[:, None, None]
    t = np.arange(TS)[None, :, None]
    l = np.arange(TS)[None, None, :]
    return pad[d * TS + l - t + TS - 1]


def _q8(a):
    return np.asarray(a, np.float32).astype(E4)


def _prepare(x, phi, M_phi_plus, M_phi_minus):
    x = np.asarray(x, np.float64)
    phi = np.asarray(phi, np.float64)
    Mp = np.asarray(M_phi_plus, np.float64)
    Mm = np.asarray(M_phi_minus, np.float64)

    # [cp, i', (b, ic, t)] x^T tiles, fp16
    xT = np.ascontiguousarray(
        x.reshape(B, CT, TS, 2, TS).transpose(1, 4, 0, 3, 2)
    ).reshape(CT, TS, B * 2 * TS).astype(np.float16)
    sgn = ((-1.0) ** np.arange(L))

    tb_all = np.empty((2 * K, CT, TS, TS), np.float64)
    m_all = np.empty((2 * K, 2, TS, O), np.float64)
    for g in range(2 * K):
        k, sg = g // 2, g % 2
        m_all[g] = (Mm if sg else Mp)[k].reshape(2, TS, O) * 64.0
        tb_all[g] = _build_toeplitz(phi[:, k] * (sgn if sg else 1.0))

    # fused s1 rhs: [pp, ic, i', (g0 o | g1 o)]
    m_fused = np.concatenate([m_all[0::2], m_all[1::2]], axis=3)

    nc = _build_nc()
    in_maps = []
    for core in range(N_CORES):
        glist = [core * NP + j for j in range(NP)]
        pps = slice(core * NPP, (core + 1) * NPP)

        # dense T lhsT: [d, t, (p, l)] = 4 * T in fp16
        t16 = np.zeros((2, TS, NP * TS), np.float64)
        for d in range(2):
            for j, g in enumerate(glist):
                t16[d, :, j * TS:(j + 1) * TS] = 4.0 * tb_all[g, d]

        # proj bases + far G (fit against quantized P8)
        p8 = np.zeros((TS, 2, TS), np.float64)
        G8 = np.zeros((CT - 2, TS, TS), np.float64)  # D-2 -> [wrow, l] halves
        for j, g in enumerate(glist):
            r = RANKS[j]
            pp, p = j // 2, j % 2
            fam = tb_all[g, 2:]
            gram = np.einsum('dtl,dsl->ts', fam, fam)
            _, vec = np.linalg.eigh(gram)
            P = np.ascontiguousarray(vec[:, ::-1][:, :r])
            P8 = _q8(S_P * P).astype(np.float64)
            col0 = SOFF[pp] + p * (SW[pp] // 2)
            p8[:, p, col0:col0 + r] = P8
            # rows in W for this pair
            wrow = SOFF[pp] + p * (SW[pp] // 2)
            A = np.linalg.solve(P8.T @ P8, P8.T)      # lsq fit vs P8
            Gfit = np.einsum('rt,dtl->drl', A, fam) * S_GF
            for D in range(2, CT):
                G8[D - 2, wrow:wrow + r, :] = Gfit[D - 2]
        # pair-lhsT tiles: idx0 = [G2; 0]; idx D-2 = [G_D; G_{D-1}]
        g8 = np.zeros((14, TS, 2, TS), np.float64)
        g8[0, :, 0, :] = G8[0]
        for D in range(3, CT):
            g8[D - 2, :, 0, :] = G8[D - 2]
            g8[D - 2, :, 1, :] = G8[D - 3]

        in_maps.append({
            "x16": xT,
            "m16": np.ascontiguousarray(m_fused[pps]).astype(np.float16),
            "t16": t16.astype(np.float16),
            "p8": _q8(p8.reshape(TS, 2 * TS)),
            "g8": _q8(g8.reshape(14, TS, 2 * TS)),
        })
    return nc, in_maps


def _gather(results):
    y = np.zeros((CT, TS, B, O), np.float64)
    for core in range(N_CORES):
        y += results[core]["yp"].reshape(CT, TS, B, O).astype(np.float64)
    return np.ascontiguousarray(
        y.transpose(2, 0, 1, 3).reshape(B, L, O)
    ).astype(np.float32)


def kernel(x, phi, M_phi_plus, M_phi_minus):
    nc, in_maps = _prepare(x, phi, M_phi_plus, M_phi_minus)
    res = run_bass_kernel_spmd(nc, in_maps, list(range(N_CORES)))
    return _gather(res.results)
